# revision 13
# baseline (speedup 1.0000x reference)
"""ArcFace loss kernel for 8 TRN2 NeuronCores (partial-FC class sharding).

Per core i of 8:
  - inputs (host-prepped layouts): emb_t [512,1024] f32 = normalized
    embeddings transposed (replicated), w_t [512,6250] f32 = normalized
    weight shard transposed (classes i*6250 ... (i+1)*6250).
  - load both into SBUF as [128, 4*cols] k-chunk-major tiles, rounding
    to f32r (TensorE's full-rate 4-byte matmul dtype) with one vector
    copy per column chunk.
  - cosine shard = emb_n @ w_n^T via f32r matmuls: out [128b, 512c]
    PSUM tiles, K=512 accumulated over 4 chunks of 128; per [128,1024]
    PSUM group the epilogue writes 64*cosine -> HBM (second reference
    output) and exp(64*cosine) row-sums -> local softmax partials
    (fused ScalarE exp+accumulate; no max-shift needed: |64*cos| <= 64
    keeps exp within f32 range).
  - each core returns its [1024] partial normalizer sums; the host adds
    the 8 partials (a collective-free partial-FC softmax reduction).
Host: l2-normalize embeddings, fold 1/||w_c|| into the weight shard,
pre-transpose both (device-friendly weight layout), and apply the O(B)
ArcFace margin correction for the label column using the returned
matrix: loss = mean(log(Z_corr) - 64*phi).
"""

import math
import os

import numpy as np

DBG_F32_MM = os.environ.get("DBG_F32_MM", "0") == "1"

B, D, C = 1024, 512, 50000
N_CORES = 8
CL = C // N_CORES  # 6250 classes per core
SCALE = 64.0
MARGIN = 0.5
COS_M = math.cos(MARGIN)
SIN_M = math.sin(MARGIN)
TH = math.cos(math.pi - MARGIN)
MM = math.sin(math.pi - MARGIN) * MARGIN

P = 128
NB = B // P  # 8 batch tiles
NK = D // P  # 4 contraction chunks
# class-dim epilogue groups (PSUM-resident width per group, <=1024 = 2 banks)
EGROUPS = [(i * 1024, 1024) for i in range(CL // 1024)] + [(CL - CL % 1024, CL % 1024)]

_CACHE = {}


def _build():
    import concourse.bass as bass  # noqa: F401
    import concourse.mybir as mybir
    import concourse.tile as tile
    from concourse import bacc

    f32 = mybir.dt.float32
    f32r = mybir.dt.float32 if DBG_F32_MM else mybir.dt.float32r

    nc = bacc.Bacc("TRN2", target_bir_lowering=False, debug=False,
                   num_devices=N_CORES)
    emb_d = nc.dram_tensor("emb_t", [D, B], f32r, kind="ExternalInput")
    w_d = nc.dram_tensor("w_t", [D, CL], f32r, kind="ExternalInput")
    out_cos = nc.dram_tensor("out_cos", [B, CL], f32, kind="ExternalOutput")
    out_z = nc.dram_tensor("out_z", [P, NB], f32, kind="ExternalOutput")

    with tile.TileContext(nc) as tc:
        with tc.tile_pool(name="persist", bufs=1) as persist, \
             tc.tile_pool(name="stage", bufs=3) as stage, \
             tc.tile_pool(name="pmm", bufs=4, space="PSUM") as pmm_pool:

            # k-chunk k of the transposed emb lives at column offset k*B
            embT = persist.tile([P, NK * B], f32r, tag="embT")
            zparts = [persist.tile([P, len(EGROUPS)], f32, tag=f"zp_{bt}",
                                   name=f"zp_{bt}")
                      for bt in range(NB)]
            z_all = persist.tile([P, NB], f32, tag="z_all")

            # ---- emb straight into its f32r tile (the PE rounds f32r
            # ---- operands internally)
            for k in range(NK):
                nc.sync.dma_start(embT[:, k * B:(k + 1) * B],
                                  emb_d.ap()[k * P:(k + 1) * P, :])

            # ---- cosine matmuls + epilogue; the weight shard streams
            # ---- through a 2-deep pool so group g+2's DMA only starts
            # ---- once group g is consumed (incremental delivery keeps
            # ---- TensorE fed instead of one all-at-once DMA wave)
            exp_scr = persist.tile([P, 1024], f32, tag="exp_scr")
            for g, (n0, n) in enumerate(EGROUPS):
                wt_g = stage.tile([P, NK * 1024], f32r, tag="wt_g", bufs=2)
                for k in range(NK):
                    nc.sync.dma_start(
                        wt_g[:, k * 1024:k * 1024 + n],
                        w_d.ap()[k * P:(k + 1) * P, n0:n0 + n])
                for bt in range(NB):
                    p_mm = pmm_pool.tile([P, 1024], f32, tag="pmm")
                    for s0 in range(0, n, 512):
                        sn = min(512, n - s0)
                        for k in range(NK):
                            nc.tensor.matmul(
                                p_mm[:, s0:s0 + sn],
                                embT[:, k * B + bt * P:k * B + (bt + 1) * P],
                                wt_g[:, k * 1024 + s0:k * 1024 + s0 + sn],
                                start=(k == 0), stop=(k == NK - 1))
                    o_t = stage.tile([P, 1024], f32, tag="o_t")
                    nc.vector.tensor_scalar_mul(o_t[:, :n], p_mm[:, :n], SCALE)
                    nc.sync.dma_start(
                        out_cos.ap()[bt * P:(bt + 1) * P, n0:n0 + n],
                        o_t[:, :n])
                    nc.scalar.activation(exp_scr[:, :n], p_mm[:, :n],
                                         mybir.ActivationFunctionType.Exp,
                                         bias=0.0, scale=SCALE,
                                         accum_out=zparts[bt][:, g:g + 1])

            # ---- local softmax normalizer partials ----
            for bt in range(NB):
                nc.vector.tensor_reduce(z_all[:, bt:bt + 1], zparts[bt][:],
                                        mybir.AxisListType.X,
                                        mybir.AluOpType.add)
            nc.sync.dma_start(out_z.ap()[:], z_all[:])

    nc.compile()
    return nc


def _get_nc():
    if "nc" not in _CACHE:
        _CACHE["nc"] = _build()
    return _CACHE["nc"]


def run_device(embeddings: np.ndarray, weight: np.ndarray, trace: bool = False):
    """Run the 8-core NEFF. Returns (cos64 [B,C] f32, Z [B] f64, results)."""
    from concourse import bass_utils

    nc = _get_nc()
    emb = np.asarray(embeddings, dtype=np.float32)
    w = np.asarray(weight, dtype=np.float32)

    # host prep: l2-normalize embeddings, fold 1/||w_c|| into the weight
    # rows, and pre-transpose both into the device layout
    emb_n = emb / np.maximum(np.linalg.norm(emb, axis=1, keepdims=True), 1e-12)
    winv = 1.0 / np.maximum(np.linalg.norm(w, axis=1), 1e-12)
    w_n_t = np.ascontiguousarray((w * winv[:, None].astype(np.float32)).T)
    emb_t = np.ascontiguousarray(emb_n.T)

    in_maps = [
        {"emb_t": emb_t,
         "w_t": np.ascontiguousarray(w_n_t[:, i * CL:(i + 1) * CL])}
        for i in range(N_CORES)
    ]
    res = bass_utils.run_bass_kernel_spmd(
        nc, in_maps, core_ids=list(range(N_CORES)), trace=trace)
    cos64 = np.concatenate([res.results[i]["out_cos"] for i in range(N_CORES)],
                           axis=1)
    z = np.sum([res.results[i]["out_z"].T.reshape(B) for i in range(N_CORES)],
               axis=0, dtype=np.float64)
    return cos64, z, res


def kernel(embeddings: np.ndarray, labels: np.ndarray, weight: np.ndarray):
    cos64, z, _ = run_device(embeddings, weight)

    # host epilogue: ArcFace margin correction for the label column, O(B)
    lab = np.asarray(labels).astype(np.int64)
    cos_t = cos64[np.arange(B), lab].astype(np.float64) / SCALE
    sin_t = np.sqrt(np.maximum(0.0, 1.0 - cos_t * cos_t))
    phi = cos_t * COS_M - sin_t * SIN_M
    phi = np.where(cos_t > TH, phi, cos_t - MM)
    z_corr = z - np.exp(SCALE * cos_t) + np.exp(SCALE * phi)
    loss = np.mean(np.log(z_corr) - SCALE * phi)
    return np.float32(loss), cos64


# revision 14
# speedup vs baseline: 1.0686x; 1.0686x over previous
"""ArcFace loss kernel for 8 TRN2 NeuronCores (partial-FC class sharding).

Per core i of 8:
  - inputs (host-prepped layouts): emb_t [512,1024] f32 = normalized
    embeddings transposed (replicated), w_t [512,6250] f32 = normalized
    weight shard transposed (classes i*6250 ... (i+1)*6250).
  - load both into SBUF as [128, 4*cols] k-chunk-major tiles, rounding
    to f32r (TensorE's full-rate 4-byte matmul dtype) with one vector
    copy per column chunk.
  - cosine shard = emb_n @ w_n^T via f32r matmuls: out [128b, 512c]
    PSUM tiles, K=512 accumulated over 4 chunks of 128; per [128,1024]
    PSUM group the epilogue writes 64*cosine -> HBM (second reference
    output) and exp(64*cosine) row-sums -> local softmax partials
    (fused ScalarE exp+accumulate; no max-shift needed: |64*cos| <= 64
    keeps exp within f32 range).
  - each core returns its [1024] partial normalizer sums; the host adds
    the 8 partials (a collective-free partial-FC softmax reduction).
Host: l2-normalize embeddings, fold 1/||w_c|| into the weight shard,
pre-transpose both (device-friendly weight layout), and apply the O(B)
ArcFace margin correction for the label column using the returned
matrix: loss = mean(log(Z_corr) - 64*phi).
"""

import math
import os

import numpy as np

DBG_F32_MM = os.environ.get("DBG_F32_MM", "0") == "1"

B, D, C = 1024, 512, 50000
N_CORES = 8
CL = C // N_CORES  # 6250 classes per core
SCALE = 64.0
MARGIN = 0.5
COS_M = math.cos(MARGIN)
SIN_M = math.sin(MARGIN)
TH = math.cos(math.pi - MARGIN)
MM = math.sin(math.pi - MARGIN) * MARGIN

P = 128
NB = B // P  # 8 batch tiles
NK = D // P  # 4 contraction chunks
# class-dim epilogue groups (PSUM-resident width per group, <=1024 = 2 banks)
EGROUPS = [(i * 1024, 1024) for i in range(CL // 1024)] + [(CL - CL % 1024, CL % 1024)]

_CACHE = {}


def _build():
    import concourse.bass as bass  # noqa: F401
    import concourse.mybir as mybir
    import concourse.tile as tile
    from concourse import bacc

    f32 = mybir.dt.float32
    f32r = mybir.dt.float32 if DBG_F32_MM else mybir.dt.float32r

    nc = bacc.Bacc("TRN2", target_bir_lowering=False, debug=False,
                   num_devices=N_CORES)
    emb_d = nc.dram_tensor("emb_t", [D, B], f32r, kind="ExternalInput")
    w_d = nc.dram_tensor("w_t", [D, CL], f32r, kind="ExternalInput")
    out_cos = nc.dram_tensor("out_cos", [B, CL], f32, kind="ExternalOutput")
    out_z = nc.dram_tensor("out_z", [P, NB], f32, kind="ExternalOutput")

    with tile.TileContext(nc) as tc:
        with tc.tile_pool(name="persist", bufs=1) as persist, \
             tc.tile_pool(name="stage", bufs=3) as stage, \
             tc.tile_pool(name="pmm", bufs=4, space="PSUM") as pmm_pool:

            # k-chunk k of the transposed emb lives at column offset k*B
            embT = persist.tile([P, NK * B], f32r, tag="embT")
            zparts = [persist.tile([P, len(EGROUPS)], f32, tag=f"zp_{bt}",
                                   name=f"zp_{bt}")
                      for bt in range(NB)]
            z_all = persist.tile([P, NB], f32, tag="z_all")

            # ---- emb straight into its f32r tile (the PE rounds f32r
            # ---- operands internally)
            for k in range(NK):
                nc.sync.dma_start(embT[:, k * B:(k + 1) * B],
                                  emb_d.ap()[k * P:(k + 1) * P, :])

            # ---- cosine matmuls + epilogue; the weight shard streams
            # ---- through a 2-deep pool so group g+2's DMA only starts
            # ---- once group g is consumed (incremental delivery keeps
            # ---- TensorE fed instead of one all-at-once DMA wave)
            exp_scr = persist.tile([P, 1024], f32, tag="exp_scr")
            for g, (n0, n) in enumerate(EGROUPS):
                wt_g = stage.tile([P, NK * 1024], f32r, tag="wt_g", bufs=2)
                for k in range(NK):
                    for q0 in range(0, n, 256):
                        qn = min(256, n - q0)
                        nc.sync.dma_start(
                            wt_g[:, k * 1024 + q0:k * 1024 + q0 + qn],
                            w_d.ap()[k * P:(k + 1) * P, n0 + q0:n0 + q0 + qn])
                for bt in range(NB):
                    p_mm = pmm_pool.tile([P, 1024], f32, tag="pmm")
                    for s0 in range(0, n, 512):
                        sn = min(512, n - s0)
                        for k in range(NK):
                            nc.tensor.matmul(
                                p_mm[:, s0:s0 + sn],
                                embT[:, k * B + bt * P:k * B + (bt + 1) * P],
                                wt_g[:, k * 1024 + s0:k * 1024 + s0 + sn],
                                start=(k == 0), stop=(k == NK - 1))
                    o_t = stage.tile([P, 1024], f32, tag="o_t")
                    nc.vector.tensor_scalar_mul(o_t[:, :n], p_mm[:, :n], SCALE)
                    nc.sync.dma_start(
                        out_cos.ap()[bt * P:(bt + 1) * P, n0:n0 + n],
                        o_t[:, :n])
                    nc.scalar.activation(exp_scr[:, :n], p_mm[:, :n],
                                         mybir.ActivationFunctionType.Exp,
                                         bias=0.0, scale=SCALE,
                                         accum_out=zparts[bt][:, g:g + 1])

            # ---- local softmax normalizer partials ----
            for bt in range(NB):
                nc.vector.tensor_reduce(z_all[:, bt:bt + 1], zparts[bt][:],
                                        mybir.AxisListType.X,
                                        mybir.AluOpType.add)
            nc.sync.dma_start(out_z.ap()[:], z_all[:])

    nc.compile()
    return nc


def _get_nc():
    if "nc" not in _CACHE:
        _CACHE["nc"] = _build()
    return _CACHE["nc"]


def run_device(embeddings: np.ndarray, weight: np.ndarray, trace: bool = False):
    """Run the 8-core NEFF. Returns (cos64 [B,C] f32, Z [B] f64, results)."""
    from concourse import bass_utils

    nc = _get_nc()
    emb = np.asarray(embeddings, dtype=np.float32)
    w = np.asarray(weight, dtype=np.float32)

    # host prep: l2-normalize embeddings, fold 1/||w_c|| into the weight
    # rows, and pre-transpose both into the device layout
    emb_n = emb / np.maximum(np.linalg.norm(emb, axis=1, keepdims=True), 1e-12)
    winv = 1.0 / np.maximum(np.linalg.norm(w, axis=1), 1e-12)
    w_n_t = np.ascontiguousarray((w * winv[:, None].astype(np.float32)).T)
    emb_t = np.ascontiguousarray(emb_n.T)

    in_maps = [
        {"emb_t": emb_t,
         "w_t": np.ascontiguousarray(w_n_t[:, i * CL:(i + 1) * CL])}
        for i in range(N_CORES)
    ]
    res = bass_utils.run_bass_kernel_spmd(
        nc, in_maps, core_ids=list(range(N_CORES)), trace=trace)
    cos64 = np.concatenate([res.results[i]["out_cos"] for i in range(N_CORES)],
                           axis=1)
    z = np.sum([res.results[i]["out_z"].T.reshape(B) for i in range(N_CORES)],
               axis=0, dtype=np.float64)
    return cos64, z, res


def kernel(embeddings: np.ndarray, labels: np.ndarray, weight: np.ndarray):
    cos64, z, _ = run_device(embeddings, weight)

    # host epilogue: ArcFace margin correction for the label column, O(B)
    lab = np.asarray(labels).astype(np.int64)
    cos_t = cos64[np.arange(B), lab].astype(np.float64) / SCALE
    sin_t = np.sqrt(np.maximum(0.0, 1.0 - cos_t * cos_t))
    phi = cos_t * COS_M - sin_t * SIN_M
    phi = np.where(cos_t > TH, phi, cos_t - MM)
    z_corr = z - np.exp(SCALE * cos_t) + np.exp(SCALE * phi)
    loss = np.mean(np.log(z_corr) - SCALE * phi)
    return np.float32(loss), cos64


# revision 15
# speedup vs baseline: 1.0820x; 1.0125x over previous
"""ArcFace loss kernel for 8 TRN2 NeuronCores (partial-FC class sharding).

Per core i of 8:
  - inputs (host-prepped layouts): emb_t [512,1024] f32 = normalized
    embeddings transposed (replicated), w_t [512,6250] f32 = normalized
    weight shard transposed (classes i*6250 ... (i+1)*6250).
  - load both into SBUF as [128, 4*cols] k-chunk-major tiles, rounding
    to f32r (TensorE's full-rate 4-byte matmul dtype) with one vector
    copy per column chunk.
  - cosine shard = emb_n @ w_n^T via f32r matmuls: out [128b, 512c]
    PSUM tiles, K=512 accumulated over 4 chunks of 128; per [128,1024]
    PSUM group the epilogue writes 64*cosine -> HBM (second reference
    output) and exp(64*cosine) row-sums -> local softmax partials
    (fused ScalarE exp+accumulate; no max-shift needed: |64*cos| <= 64
    keeps exp within f32 range).
  - each core returns its [1024] partial normalizer sums; the host adds
    the 8 partials (a collective-free partial-FC softmax reduction).
Host: l2-normalize embeddings, fold 1/||w_c|| into the weight shard,
pre-transpose both (device-friendly weight layout), and apply the O(B)
ArcFace margin correction for the label column using the returned
matrix: loss = mean(log(Z_corr) - 64*phi).
"""

import math
import os

import numpy as np

DBG_F32_MM = os.environ.get("DBG_F32_MM", "0") == "1"

B, D, C = 1024, 512, 50000
N_CORES = 8
CL = C // N_CORES  # 6250 classes per core
SCALE = 64.0
MARGIN = 0.5
COS_M = math.cos(MARGIN)
SIN_M = math.sin(MARGIN)
TH = math.cos(math.pi - MARGIN)
MM = math.sin(math.pi - MARGIN) * MARGIN

P = 128
NB = B // P  # 8 batch tiles
NK = D // P  # 4 contraction chunks
# class-dim epilogue groups (PSUM-resident width per group, <=1024 = 2 banks)
EGROUPS = [(i * 1024, 1024) for i in range(CL // 1024)] + [(CL - CL % 1024, CL % 1024)]

_CACHE = {}


def _build():
    import concourse.bass as bass  # noqa: F401
    import concourse.mybir as mybir
    import concourse.tile as tile
    from concourse import bacc

    f32 = mybir.dt.float32
    f32r = mybir.dt.float32 if DBG_F32_MM else mybir.dt.float32r

    nc = bacc.Bacc("TRN2", target_bir_lowering=False, debug=False,
                   num_devices=N_CORES)
    emb_d = nc.dram_tensor("emb_t", [D, B], f32r, kind="ExternalInput")
    w_d = nc.dram_tensor("w_t", [D, CL], f32r, kind="ExternalInput")
    out_cos = nc.dram_tensor("out_cos", [B, CL], f32, kind="ExternalOutput")
    out_z = nc.dram_tensor("out_z", [P, NB], f32, kind="ExternalOutput")

    with tile.TileContext(nc) as tc:
        with tc.tile_pool(name="persist", bufs=1) as persist, \
             tc.tile_pool(name="stage", bufs=3) as stage, \
             tc.tile_pool(name="pmm", bufs=4, space="PSUM") as pmm_pool:

            # k-chunk k of the transposed emb lives at column offset k*B
            embT = persist.tile([P, NK * B], f32r, tag="embT")
            zparts = [persist.tile([P, len(EGROUPS)], f32, tag=f"zp_{bt}",
                                   name=f"zp_{bt}")
                      for bt in range(NB)]
            z_all = persist.tile([P, NB], f32, tag="z_all")

            # ---- emb straight into its f32r tile (the PE rounds f32r
            # ---- operands internally)
            for k in range(NK):
                nc.sync.dma_start(embT[:, k * B:(k + 1) * B],
                                  emb_d.ap()[k * P:(k + 1) * P, :])

            # ---- cosine matmuls + epilogue; the weight shard streams
            # ---- through a 2-deep pool so group g+2's DMA only starts
            # ---- once group g is consumed (incremental delivery keeps
            # ---- TensorE fed instead of one all-at-once DMA wave)
            exp_scr = persist.tile([P, 1024], f32, tag="exp_scr")
            for g, (n0, n) in enumerate(EGROUPS):
                wt_g = stage.tile([P, NK * 1024], f32r, tag="wt_g", bufs=4)
                for k in range(NK):
                    for q0 in range(0, n, 256):
                        qn = min(256, n - q0)
                        nc.sync.dma_start(
                            wt_g[:, k * 1024 + q0:k * 1024 + q0 + qn],
                            w_d.ap()[k * P:(k + 1) * P, n0 + q0:n0 + q0 + qn])
                for bt in range(NB):
                    p_mm = pmm_pool.tile([P, 1024], f32, tag="pmm")
                    for s0 in range(0, n, 512):
                        sn = min(512, n - s0)
                        for k in range(NK):
                            nc.tensor.matmul(
                                p_mm[:, s0:s0 + sn],
                                embT[:, k * B + bt * P:k * B + (bt + 1) * P],
                                wt_g[:, k * 1024 + s0:k * 1024 + s0 + sn],
                                start=(k == 0), stop=(k == NK - 1))
                    o_t = stage.tile([P, 1024], f32, tag="o_t")
                    nc.vector.tensor_scalar_mul(o_t[:, :n], p_mm[:, :n], SCALE)
                    nc.sync.dma_start(
                        out_cos.ap()[bt * P:(bt + 1) * P, n0:n0 + n],
                        o_t[:, :n])
                    nc.scalar.activation(exp_scr[:, :n], p_mm[:, :n],
                                         mybir.ActivationFunctionType.Exp,
                                         bias=0.0, scale=SCALE,
                                         accum_out=zparts[bt][:, g:g + 1])

            # ---- local softmax normalizer partials ----
            for bt in range(NB):
                nc.vector.tensor_reduce(z_all[:, bt:bt + 1], zparts[bt][:],
                                        mybir.AxisListType.X,
                                        mybir.AluOpType.add)
            nc.sync.dma_start(out_z.ap()[:], z_all[:])

    nc.compile()
    return nc


def _get_nc():
    if "nc" not in _CACHE:
        _CACHE["nc"] = _build()
    return _CACHE["nc"]


def run_device(embeddings: np.ndarray, weight: np.ndarray, trace: bool = False):
    """Run the 8-core NEFF. Returns (cos64 [B,C] f32, Z [B] f64, results)."""
    from concourse import bass_utils

    nc = _get_nc()
    emb = np.asarray(embeddings, dtype=np.float32)
    w = np.asarray(weight, dtype=np.float32)

    # host prep: l2-normalize embeddings, fold 1/||w_c|| into the weight
    # rows, and pre-transpose both into the device layout
    emb_n = emb / np.maximum(np.linalg.norm(emb, axis=1, keepdims=True), 1e-12)
    winv = 1.0 / np.maximum(np.linalg.norm(w, axis=1), 1e-12)
    w_n_t = np.ascontiguousarray((w * winv[:, None].astype(np.float32)).T)
    emb_t = np.ascontiguousarray(emb_n.T)

    in_maps = [
        {"emb_t": emb_t,
         "w_t": np.ascontiguousarray(w_n_t[:, i * CL:(i + 1) * CL])}
        for i in range(N_CORES)
    ]
    res = bass_utils.run_bass_kernel_spmd(
        nc, in_maps, core_ids=list(range(N_CORES)), trace=trace)
    cos64 = np.concatenate([res.results[i]["out_cos"] for i in range(N_CORES)],
                           axis=1)
    z = np.sum([res.results[i]["out_z"].T.reshape(B) for i in range(N_CORES)],
               axis=0, dtype=np.float64)
    return cos64, z, res


def kernel(embeddings: np.ndarray, labels: np.ndarray, weight: np.ndarray):
    cos64, z, _ = run_device(embeddings, weight)

    # host epilogue: ArcFace margin correction for the label column, O(B)
    lab = np.asarray(labels).astype(np.int64)
    cos_t = cos64[np.arange(B), lab].astype(np.float64) / SCALE
    sin_t = np.sqrt(np.maximum(0.0, 1.0 - cos_t * cos_t))
    phi = cos_t * COS_M - sin_t * SIN_M
    phi = np.where(cos_t > TH, phi, cos_t - MM)
    z_corr = z - np.exp(SCALE * cos_t) + np.exp(SCALE * phi)
    loss = np.mean(np.log(z_corr) - SCALE * phi)
    return np.float32(loss), cos64


# revision 16
# speedup vs baseline: 1.2349x; 1.1413x over previous
"""ArcFace loss kernel for 8 TRN2 NeuronCores (partial-FC class sharding).

Per core i of 8:
  - inputs (host-prepped layouts): emb_t [512,1024] f32 = normalized
    embeddings transposed (replicated), w_t [512,6250] f32 = normalized
    weight shard transposed (classes i*6250 ... (i+1)*6250).
  - load both into SBUF as [128, 4*cols] k-chunk-major tiles, rounding
    to f32r (TensorE's full-rate 4-byte matmul dtype) with one vector
    copy per column chunk.
  - cosine shard = emb_n @ w_n^T via f32r matmuls: out [128b, 512c]
    PSUM tiles, K=512 accumulated over 4 chunks of 128; per [128,1024]
    PSUM group the epilogue writes 64*cosine -> HBM (second reference
    output) and exp(64*cosine) row-sums -> local softmax partials
    (fused ScalarE exp+accumulate; no max-shift needed: |64*cos| <= 64
    keeps exp within f32 range).
  - each core returns its [1024] partial normalizer sums; the host adds
    the 8 partials (a collective-free partial-FC softmax reduction).
Host: l2-normalize embeddings, fold 1/||w_c|| into the weight shard,
pre-transpose both (device-friendly weight layout), and apply the O(B)
ArcFace margin correction for the label column using the returned
matrix: loss = mean(log(Z_corr) - 64*phi).
"""

import math
import os

import numpy as np

MM_DTYPE = os.environ.get("MM_DTYPE", "f32r")

B, D, C = 1024, 512, 50000
N_CORES = 8
CL = C // N_CORES  # 6250 classes per core
SCALE = 64.0
MARGIN = 0.5
COS_M = math.cos(MARGIN)
SIN_M = math.sin(MARGIN)
TH = math.cos(math.pi - MARGIN)
MM = math.sin(math.pi - MARGIN) * MARGIN

P = 128
NB = B // P  # 8 batch tiles
NK = D // P  # 4 contraction chunks
# class-dim epilogue groups (PSUM-resident width per group, <=1024 = 2 banks)
EGROUPS = [(i * 1024, 1024) for i in range(CL // 1024)] + [(CL - CL % 1024, CL % 1024)]

_CACHE = {}


def _build():
    import concourse.bass as bass  # noqa: F401
    import concourse.mybir as mybir
    import concourse.tile as tile
    from concourse import bacc

    f32 = mybir.dt.float32
    f32r = {"f32r": mybir.dt.float32r, "bf16": mybir.dt.bfloat16,
            "f32": mybir.dt.float32}[MM_DTYPE]

    nc = bacc.Bacc("TRN2", target_bir_lowering=False, debug=False,
                   num_devices=N_CORES)
    emb_d = nc.dram_tensor("emb_t", [D, B], f32r, kind="ExternalInput")
    w_d = nc.dram_tensor("w_t", [D, CL], f32r, kind="ExternalInput")
    out_cos = nc.dram_tensor("out_cos", [B, CL], f32, kind="ExternalOutput")
    out_z = nc.dram_tensor("out_z", [P, NB], f32, kind="ExternalOutput")

    with tile.TileContext(nc) as tc:
        with tc.tile_pool(name="persist", bufs=1) as persist, \
             tc.tile_pool(name="stage", bufs=3) as stage, \
             tc.tile_pool(name="pmm", bufs=4, space="PSUM") as pmm_pool:

            # k-chunk k of the transposed operands lives at column offset
            # k*B / k*CL of one wide tile.
            embT = persist.tile([P, NK * B], f32r, tag="embT")
            wT = persist.tile([P, NK * CL], f32r, tag="wT")
            zparts = [persist.tile([P, len(EGROUPS)], f32, tag=f"zp_{bt}",
                                   name=f"zp_{bt}")
                      for bt in range(NB)]
            z_all = persist.tile([P, NB], f32, tag="z_all")

            # ---- load both operands straight into matmul-dtype tiles
            # ---- (the PE rounds f32r operands internally)
            for k in range(NK):
                nc.sync.dma_start(embT[:, k * B:(k + 1) * B],
                                  emb_d.ap()[k * P:(k + 1) * P, :])
            for g, (n0, n) in enumerate(EGROUPS):
                for k in range(NK):
                    nc.sync.dma_start(
                        wT[:, k * CL + n0:k * CL + n0 + n],
                        w_d.ap()[k * P:(k + 1) * P, n0:n0 + n])

            # ---- cosine matmuls + epilogue ----
            exp_scr = persist.tile([P, 1024], f32, tag="exp_scr")
            for g, (n0, n) in enumerate(EGROUPS):
                for bt in range(NB):
                    p_mm = pmm_pool.tile([P, 1024], f32, tag="pmm")
                    for s0 in range(0, n, 512):
                        sn = min(512, n - s0)
                        for k in range(NK):
                            nc.tensor.matmul(
                                p_mm[:, s0:s0 + sn],
                                embT[:, k * B + bt * P:k * B + (bt + 1) * P],
                                wT[:, k * CL + n0 + s0:k * CL + n0 + s0 + sn],
                                start=(k == 0), stop=(k == NK - 1))
                    o_t = stage.tile([P, 1024], f32, tag="o_t")
                    nc.vector.tensor_scalar_mul(o_t[:, :n], p_mm[:, :n], SCALE)
                    nc.sync.dma_start(
                        out_cos.ap()[bt * P:(bt + 1) * P, n0:n0 + n],
                        o_t[:, :n])
                    nc.scalar.activation(exp_scr[:, :n], p_mm[:, :n],
                                         mybir.ActivationFunctionType.Exp,
                                         bias=0.0, scale=SCALE,
                                         accum_out=zparts[bt][:, g:g + 1])

            # ---- local softmax normalizer partials ----
            for bt in range(NB):
                nc.vector.tensor_reduce(z_all[:, bt:bt + 1], zparts[bt][:],
                                        mybir.AxisListType.X,
                                        mybir.AluOpType.add)
            nc.sync.dma_start(out_z.ap()[:], z_all[:])

    nc.compile()
    return nc


def _get_nc():
    if "nc" not in _CACHE:
        _CACHE["nc"] = _build()
    return _CACHE["nc"]


def run_device(embeddings: np.ndarray, weight: np.ndarray, trace: bool = False):
    """Run the 8-core NEFF. Returns (cos64 [B,C] f32, Z [B] f64, results)."""
    from concourse import bass_utils

    nc = _get_nc()
    emb = np.asarray(embeddings, dtype=np.float32)
    w = np.asarray(weight, dtype=np.float32)

    # host prep: l2-normalize embeddings, fold 1/||w_c|| into the weight
    # rows, and pre-transpose both into the device layout
    emb_n = emb / np.maximum(np.linalg.norm(emb, axis=1, keepdims=True), 1e-12)
    winv = 1.0 / np.maximum(np.linalg.norm(w, axis=1), 1e-12)
    w_n_t = np.ascontiguousarray((w * winv[:, None].astype(np.float32)).T)
    emb_t = np.ascontiguousarray(emb_n.T)

    if MM_DTYPE == "bf16":
        import ml_dtypes
        emb_t = emb_t.astype(ml_dtypes.bfloat16)
        w_n_t = w_n_t.astype(ml_dtypes.bfloat16)
    in_maps = [
        {"emb_t": emb_t,
         "w_t": np.ascontiguousarray(w_n_t[:, i * CL:(i + 1) * CL])}
        for i in range(N_CORES)
    ]
    res = bass_utils.run_bass_kernel_spmd(
        nc, in_maps, core_ids=list(range(N_CORES)), trace=trace)
    cos64 = np.concatenate([res.results[i]["out_cos"] for i in range(N_CORES)],
                           axis=1)
    z = np.sum([res.results[i]["out_z"].T.reshape(B) for i in range(N_CORES)],
               axis=0, dtype=np.float64)
    return cos64, z, res


def kernel(embeddings: np.ndarray, labels: np.ndarray, weight: np.ndarray):
    cos64, z, _ = run_device(embeddings, weight)

    # host epilogue: ArcFace margin correction for the label column, O(B)
    lab = np.asarray(labels).astype(np.int64)
    cos_t = cos64[np.arange(B), lab].astype(np.float64) / SCALE
    sin_t = np.sqrt(np.maximum(0.0, 1.0 - cos_t * cos_t))
    phi = cos_t * COS_M - sin_t * SIN_M
    phi = np.where(cos_t > TH, phi, cos_t - MM)
    z_corr = z - np.exp(SCALE * cos_t) + np.exp(SCALE * phi)
    loss = np.mean(np.log(z_corr) - SCALE * phi)
    return np.float32(loss), cos64


# revision 17
# speedup vs baseline: 1.2865x; 1.0417x over previous
"""ArcFace loss kernel for 8 TRN2 NeuronCores (partial-FC class sharding).

Per core i of 8:
  - inputs (host-prepped layouts): emb_t [512,1024] f32 = normalized
    embeddings transposed (replicated), w_t [512,6250] f32 = normalized
    weight shard transposed (classes i*6250 ... (i+1)*6250).
  - load both into SBUF as [128, 4*cols] k-chunk-major tiles, rounding
    to f32r (TensorE's full-rate 4-byte matmul dtype) with one vector
    copy per column chunk.
  - cosine shard = emb_n @ w_n^T via f32r matmuls: out [128b, 512c]
    PSUM tiles, K=512 accumulated over 4 chunks of 128; per [128,1024]
    PSUM group the epilogue writes 64*cosine -> HBM (second reference
    output) and exp(64*cosine) row-sums -> local softmax partials
    (fused ScalarE exp+accumulate; no max-shift needed: |64*cos| <= 64
    keeps exp within f32 range).
  - each core returns its [1024] partial normalizer sums; the host adds
    the 8 partials (a collective-free partial-FC softmax reduction).
Host: l2-normalize embeddings, fold 1/||w_c|| into the weight shard,
pre-transpose both (device-friendly weight layout), and apply the O(B)
ArcFace margin correction for the label column using the returned
matrix: loss = mean(log(Z_corr) - 64*phi).
"""

import math
import os

import numpy as np

MM_DTYPE = os.environ.get("MM_DTYPE", "bf16")

B, D, C = 1024, 512, 50000
N_CORES = 8
CL = C // N_CORES  # 6250 classes per core
SCALE = 64.0
MARGIN = 0.5
COS_M = math.cos(MARGIN)
SIN_M = math.sin(MARGIN)
TH = math.cos(math.pi - MARGIN)
MM = math.sin(math.pi - MARGIN) * MARGIN

P = 128
NB = B // P  # 8 batch tiles
NK = D // P  # 4 contraction chunks
# class-dim epilogue groups (PSUM-resident width per group, <=1024 = 2 banks)
EGROUPS = [(i * 1024, 1024) for i in range(CL // 1024)] + [(CL - CL % 1024, CL % 1024)]

_CACHE = {}


def _build():
    import concourse.bass as bass  # noqa: F401
    import concourse.mybir as mybir
    import concourse.tile as tile
    from concourse import bacc

    f32 = mybir.dt.float32
    f32r = {"f32r": mybir.dt.float32r, "bf16": mybir.dt.bfloat16,
            "f32": mybir.dt.float32}[MM_DTYPE]

    nc = bacc.Bacc("TRN2", target_bir_lowering=False, debug=False,
                   num_devices=N_CORES)
    emb_d = nc.dram_tensor("emb_t", [D, B], f32r, kind="ExternalInput")
    w_d = nc.dram_tensor("w_t", [D, CL], f32r, kind="ExternalInput")
    out_cos = nc.dram_tensor("out_cos", [B, CL], f32, kind="ExternalOutput")
    out_z = nc.dram_tensor("out_z", [P, NB], f32, kind="ExternalOutput")

    with tile.TileContext(nc) as tc:
        with tc.tile_pool(name="persist", bufs=1) as persist, \
             tc.tile_pool(name="stage", bufs=3) as stage, \
             tc.tile_pool(name="pmm", bufs=4, space="PSUM") as pmm_pool:

            # k-chunk k of the transposed operands lives at column offset
            # k*B / k*CL of one wide tile.
            embT = persist.tile([P, NK * B], f32r, tag="embT")
            wT = persist.tile([P, NK * CL], f32r, tag="wT")
            zparts = [persist.tile([P, len(EGROUPS)], f32, tag=f"zp_{bt}",
                                   name=f"zp_{bt}")
                      for bt in range(NB)]
            z_all = persist.tile([P, NB], f32, tag="z_all")

            # ---- load both operands straight into matmul-dtype tiles
            # ---- (the PE rounds f32r operands internally)
            for k in range(NK):
                nc.sync.dma_start(embT[:, k * B:(k + 1) * B],
                                  emb_d.ap()[k * P:(k + 1) * P, :])
            for g, (n0, n) in enumerate(EGROUPS):
                for k in range(NK):
                    nc.sync.dma_start(
                        wT[:, k * CL + n0:k * CL + n0 + n],
                        w_d.ap()[k * P:(k + 1) * P, n0:n0 + n])

            # ---- cosine matmuls + epilogue ----
            exp_scr = persist.tile([P, 1024], f32, tag="exp_scr")
            for g, (n0, n) in enumerate(EGROUPS):
                for bt in range(NB):
                    p_mm = pmm_pool.tile([P, 1024], f32, tag="pmm")
                    for s0 in range(0, n, 512):
                        sn = min(512, n - s0)
                        for k in range(NK):
                            nc.tensor.matmul(
                                p_mm[:, s0:s0 + sn],
                                embT[:, k * B + bt * P:k * B + (bt + 1) * P],
                                wT[:, k * CL + n0 + s0:k * CL + n0 + s0 + sn],
                                start=(k == 0), stop=(k == NK - 1))
                    o_t = stage.tile([P, 1024], f32, tag="o_t")
                    nc.vector.tensor_scalar_mul(o_t[:, :n], p_mm[:, :n], SCALE)
                    nc.sync.dma_start(
                        out_cos.ap()[bt * P:(bt + 1) * P, n0:n0 + n],
                        o_t[:, :n])
                    nc.scalar.activation(exp_scr[:, :n], p_mm[:, :n],
                                         mybir.ActivationFunctionType.Exp,
                                         bias=0.0, scale=SCALE,
                                         accum_out=zparts[bt][:, g:g + 1])

            # ---- local softmax normalizer partials ----
            for bt in range(NB):
                nc.vector.tensor_reduce(z_all[:, bt:bt + 1], zparts[bt][:],
                                        mybir.AxisListType.X,
                                        mybir.AluOpType.add)
            nc.sync.dma_start(out_z.ap()[:], z_all[:])

    nc.compile()
    return nc


def _get_nc():
    if "nc" not in _CACHE:
        _CACHE["nc"] = _build()
    return _CACHE["nc"]


def run_device(embeddings: np.ndarray, weight: np.ndarray, trace: bool = False):
    """Run the 8-core NEFF. Returns (cos64 [B,C] f32, Z [B] f64, results)."""
    from concourse import bass_utils

    nc = _get_nc()
    emb = np.asarray(embeddings, dtype=np.float32)
    w = np.asarray(weight, dtype=np.float32)

    # host prep: l2-normalize embeddings, fold 1/||w_c|| into the weight
    # rows, and pre-transpose both into the device layout
    emb_n = emb / np.maximum(np.linalg.norm(emb, axis=1, keepdims=True), 1e-12)
    winv = 1.0 / np.maximum(np.linalg.norm(w, axis=1), 1e-12)
    w_n_t = np.ascontiguousarray((w * winv[:, None].astype(np.float32)).T)
    emb_t = np.ascontiguousarray(emb_n.T)

    if MM_DTYPE == "bf16":
        import ml_dtypes
        emb_t = emb_t.astype(ml_dtypes.bfloat16)
        w_n_t = w_n_t.astype(ml_dtypes.bfloat16)
    in_maps = [
        {"emb_t": emb_t,
         "w_t": np.ascontiguousarray(w_n_t[:, i * CL:(i + 1) * CL])}
        for i in range(N_CORES)
    ]
    res = bass_utils.run_bass_kernel_spmd(
        nc, in_maps, core_ids=list(range(N_CORES)), trace=trace)
    cos64 = np.concatenate([res.results[i]["out_cos"] for i in range(N_CORES)],
                           axis=1)
    z = np.sum([res.results[i]["out_z"].T.reshape(B) for i in range(N_CORES)],
               axis=0, dtype=np.float64)
    return cos64, z, res


def kernel(embeddings: np.ndarray, labels: np.ndarray, weight: np.ndarray):
    cos64, z, _ = run_device(embeddings, weight)

    # host epilogue: ArcFace margin correction for the label column, O(B)
    lab = np.asarray(labels).astype(np.int64)
    cos_t = cos64[np.arange(B), lab].astype(np.float64) / SCALE
    sin_t = np.sqrt(np.maximum(0.0, 1.0 - cos_t * cos_t))
    phi = cos_t * COS_M - sin_t * SIN_M
    phi = np.where(cos_t > TH, phi, cos_t - MM)
    z_corr = z - np.exp(SCALE * cos_t) + np.exp(SCALE * phi)
    loss = np.mean(np.log(z_corr) - SCALE * phi)
    return np.float32(loss), cos64
